# revision 42
# baseline (speedup 1.0000x reference)
"""Multi-head attention (B=2, S=2048, D=1024, H=16) on 8 trn2 NeuronCores.

Sharding: core c -> batch b = c//4, head-group g = c%4 (4 heads each).
Each core: QKV projections for its 256 output dims, causal attention for its
4 heads, partial output projection over its 256 contraction dims.
Host: sum the 4 partial outputs per batch, add (bo + bv @ wo.T).

Device formulation (per core, all layouts transposed so no P-transpose is
ever needed):
  QT = (wqT_s.T @ xT + bq)         # [256 qdim, 2048 rows] on chip
  KT = same                        # [256, 2048]
  V  = natural [2048 rows, 256] with an appended ones column per head
  sT[k,q] = sum_d KT[d,k] QT[d,q]  -> psum [128k, 512q] tiles; the two heads
      of a pair run as concurrent row-tiles (lhsT base partitions 0/64)
  mask: additive -1e9 on mixed 128x128 blocks (from the real mask input)
  P = exp(sT/8)  (no max subtraction; scores are O(5) so exp is safe and
      softmax is shift-invariant)
  [OT; l] = [V|1].T @ P            # psum [65, 512q]; row 64 = denominator
  OT_norm = OT * (1/l)             # 1/l replicated over partitions via a
                                   # K=2 indicator matmul, then DVE mult
  yT_partial = woT_s.T @ OT_norm   # [1024, 2048] fp16 -> DRAM

PSUM budget (8 banks x 2KB):
  ss: scores pairs [128,2,512] f32, bufs=2  -> 4 banks
  pp: proj halves / rl-replicate / outproj [<=128,512] f32, bufs=2 -> 2
  po: [65,512] PV accumulators, bufs=2      -> 2
Separate tags let the run-ahead Tile scheduler fill the ACT(exp)-bound
attention bubbles with projection matmuls instead of stalling on psum.
"""

import os
import hashlib
import numpy as np

B, S, D, H, DK = 2, 2048, 1024, 16, 64
NCORES = 8
GROUPS = 4          # head groups per batch
HPG = 4             # heads per group (per core)
GDIM = HPG * DK     # 256 output dims per core
NEG = -1.0e9
QB = 512            # q block width
NQB = S // QB       # 4
NKT = S // 128      # 16 k tiles
NDM = D // 128      # 8 contraction tiles for projections

MODE = os.environ.get("BASS_MHA_MODE", "bf16")  # fp32 | bf16

_CACHE = {}


def _make_plan(m2d):
    """Classify 128x128 blocks of the (q,k) mask into skip/full/mixed.

    Returns per (qb, j): (j, cmin_local, bias_cols) where bias_cols is a list
    of (c_local, uniq_tile_idx); plus the packed unique bias blocks.
    """
    sub = np.asarray(m2d).reshape(S // 128, 128, S // 128, 128)
    any_ = sub.any(axis=(1, 3))   # [qtile, ktile]
    all_ = sub.all(axis=(1, 3))

    uniq = {}
    uniq_src = []
    plan = []
    for qb in range(NQB):
        entries = []
        cs = list(range(4 * qb, 4 * qb + 4))
        for j in range(NKT):
            states = []
            for c in cs:
                if not any_[c, j]:
                    states.append("skip")
                elif all_[c, j]:
                    states.append("full")
                else:
                    states.append("mixed")
            if all(s == "skip" for s in states):
                continue
            cmin = next(i for i, s in enumerate(states) if s != "skip")
            bias_cols = []
            for i in range(cmin, 4):
                if states[i] == "full":
                    continue
                c = cs[i]
                if states[i] == "skip":
                    blk = np.full((128, 128), NEG, np.float32)
                else:
                    m = sub[c, :, j, :]  # [128 q, 128 k]
                    blk = np.where(m.T != 0, 0.0, NEG).astype(np.float32)
                hsh = hashlib.sha1(blk.tobytes()).hexdigest()
                if hsh not in uniq:
                    uniq[hsh] = len(uniq_src)
                    uniq_src.append(blk)
                bias_cols.append((i, uniq[hsh]))
            entries.append((j, cmin, bias_cols))
        plan.append(entries)
    bias_pack = (
        np.stack(uniq_src) if uniq_src else np.zeros((1, 128, 128), np.float32)
    )
    key = hashlib.sha1(
        repr([(qb, e) for qb, e in enumerate(plan)]).encode()
    ).hexdigest()
    return plan, bias_pack, key


def _build(mode, plan, n_bias):
    import concourse.mybir as mybir
    from concourse import bacc, tile

    f32 = mybir.dt.float32
    f16 = mybir.dt.float16
    bf16 = mybir.dt.bfloat16
    st_dt = bf16 if mode == "bf16" else f32

    AF = mybir.ActivationFunctionType
    AO = mybir.AluOpType

    nc = bacc.Bacc(
        "TRN2", target_bir_lowering=False, debug=False, num_devices=NCORES
    )

    io_dt = bf16 if mode == "bf16" else f32
    qT_d = nc.declare_dram_parameter("qT", [D, S], io_dt, isOutput=False).ap()
    kT_d = nc.declare_dram_parameter("kT", [D, S], io_dt, isOutput=False).ap()
    vT_d = nc.declare_dram_parameter("vT", [D, S], io_dt, isOutput=False).ap()
    wqT_d = nc.declare_dram_parameter("wqT", [D, GDIM], io_dt, isOutput=False).ap()
    wkT_d = nc.declare_dram_parameter("wkT", [D, GDIM], io_dt, isOutput=False).ap()
    wvT_d = nc.declare_dram_parameter("wvT", [D, GDIM], io_dt, isOutput=False).ap()
    woT_d = nc.declare_dram_parameter("woT", [GDIM, D], io_dt, isOutput=False).ap()
    bq_d = nc.declare_dram_parameter("bq2", [128, 2], f32, isOutput=False).ap()
    bk_d = nc.declare_dram_parameter("bk2", [128, 2], f32, isOutput=False).ap()
    bias_d = nc.declare_dram_parameter(
        "bias_pack", [n_bias, 128, 128], f32, isOutput=False
    ).ap()
    yT_d = nc.declare_dram_parameter("yT", [D, S], f16, isOutput=True).ap()

    with tile.TileContext(nc) as tc:
        with (
            tc.tile_pool(name="res", bufs=1) as res,
            tc.tile_pool(name="ot_pool", bufs=4) as ot_pool,
            tc.tile_pool(name="instream", bufs=26) as instream,
            tc.tile_pool(name="ptp", bufs=6) as ptp,
            tc.tile_pool(name="park", bufs=24) as park,
            tc.tile_pool(name="ystage", bufs=6) as ystage,
            tc.tile_pool(name="small", bufs=4) as small,
            tc.tile_pool(name="psum", bufs=2, space="PSUM") as psum,
        ):
            dma = nc.sync.dma_start

            # ---- resident weights / constants (per-dm DMAs so the first
            # projection matmul can start as soon as its slice lands) ----
            wq_sb = res.tile([128, NDM, GDIM], st_dt, name="wq_sb")
            wk_sb = res.tile([128, NDM, GDIM], st_dt, name="wk_sb")
            wv_sb = res.tile([128, NDM, GDIM], st_dt, name="wv_sb")
            wo_sb = res.tile([128, 2, D], st_dt, name="wo_sb")
            bq_sb = res.tile([128, 2], f32, name="bq_sb")
            bk_sb = res.tile([128, 2], f32, name="bk_sb")
            bias_sb = res.tile([128, n_bias, 128], f32, name="bias_sb")

            QT_c = [res.tile([128, 2, 512], st_dt, name=f"QT{i}") for i in range(4)]
            KT_c = [res.tile([128, 2, 512], st_dt, name=f"KT{i}") for i in range(4)]
            V_c = [
                res.tile([128, 4, HPG, DK + 1], st_dt, name=f"V{i}")
                for i in range(4)
            ]
            ones64_sb = res.tile([1, 64], f32, name="ones64_sb")

            dma_act = nc.scalar.dma_start

            def load_w(w_sb, w_d, dm, eng=None):
                (eng or dma)(
                    out=w_sb[:, dm, :],
                    in_=w_d[128 * dm : 128 * (dm + 1), :],
                )

            def stream_x(src_d, ci, dm, tag, eng=None):
                xt = instream.tile([128, 512], st_dt, name="xt", tag="xt")
                (eng or dma)(
                    out=xt,
                    in_=src_d[
                        128 * dm : 128 * (dm + 1), 512 * ci : 512 * (ci + 1)
                    ],
                )
                return xt

            def proj_qk(ci, src_d, w_sb, b_sb, dst, xts=None):
                # Q or K projection for 512-row chunk ci (ot-outer, 1-bank psum)
                if xts is None:
                    xts = [stream_x(src_d, ci, dm, "xt") for dm in range(NDM)]
                for ot in range(2):
                    ps = psum.tile([128, 512], f32, name="ps_p", tag="pp")
                    for dm in range(NDM):
                        nc.tensor.matmul(
                            ps,
                            lhsT=w_sb[:, dm, 128 * ot : 128 * (ot + 1)],
                            rhs=xts[dm],
                            start=(dm == 0),
                            stop=(dm == NDM - 1),
                        )
                    nc.vector.tensor_scalar_add(
                        dst[:, ot, :], ps, b_sb[:, ot : ot + 1]
                    )

            def proj_v(ci, xts=None):
                if xts is None:
                    xts = [stream_x(vT_d, ci, dm, "xt") for dm in range(NDM)]
                for half in range(2):
                    ps_v = psum.tile([128, 2, GDIM], f32, name="ps_v", tag="pp")
                    for rl in range(2):
                        rt = 2 * half + rl
                        for dm in range(NDM):
                            nc.tensor.matmul(
                                ps_v[:, rl, :],
                                lhsT=xts[dm][:, 128 * rt : 128 * (rt + 1)],
                                rhs=wv_sb[:, dm, :],
                                start=(dm == 0),
                                stop=(dm == NDM - 1),
                            )
                    for rl in range(2):
                        nc.vector.tensor_copy(
                            out=V_c[ci][:, 2 * half + rl, :, 0:DK],
                            in_=ps_v[:, rl, :].rearrange(
                                "p (h d) -> p h d", d=DK
                            ),
                        )

            def score_exp(qb, pr, entry, pool, tag=None):
                # scores + mask-bias + exp for one k-tile entry -> pt in SBUF
                j, cmin, bias_cols = entry
                heads = (2 * pr, 2 * pr + 1)
                off = 128 * cmin
                jc, jl = j // 4, j % 4
                ps_s = psum.tile([128, 2, QB], f32, name="ps_s", tag="ss")
                for hh, h in enumerate(heads):
                    p0 = 64 * hh
                    ht = h // 2
                    nc.tensor.matmul(
                        ps_s[:, hh, off:QB],
                        lhsT=KT_c[jc][
                            p0 : p0 + 64, ht, 128 * jl : 128 * (jl + 1)
                        ],
                        rhs=QT_c[qb][p0 : p0 + 64, ht, off:QB],
                        start=True,
                        stop=True,
                    )
                for hh in range(2):
                    for cl, ui in bias_cols:
                        co = 128 * cl
                        nc.vector.tensor_tensor(
                            out=ps_s[:, hh, co : co + 128],
                            in0=ps_s[:, hh, co : co + 128],
                            in1=bias_sb[:, ui, :],
                            op=AO.add,
                        )
                kw = {"tag": tag} if tag else {}
                pt = pool.tile([128, 2, QB], st_dt, name="pt", **kw)
                nc.scalar.activation(
                    pt[:, :, off:QB],
                    ps_s[:, :, off:QB],
                    AF.Exp,
                    scale=0.125,
                )
                return pt, ps_s

            def attn_park(qb, pr, jmax):
                # produce exp'd score tiles early (while ACT has slack) for
                # the k-tiles of this pair that need no fresh projections
                parked = {}
                for entry in plan[qb]:
                    if entry[0] > jmax:
                        break
                    pt, _ = score_exp(qb, pr, entry, park, tag="park")
                    parked[entry[0]] = pt
                return parked

            def attn_pair(qb, pr, OT_sb, parked=None):
                entries = plan[qb]
                last_j = entries[-1][0]
                first_j = entries[0][0]
                heads = (2 * pr, 2 * pr + 1)
                po = {}
                for h in heads:
                    po[h] = psum.tile(
                        [DK + 1, QB], f32, name=f"po{h}", tag="po"
                    )
                ps_s = None
                for entry in entries:
                    j, cmin, bias_cols = entry
                    off = 128 * cmin
                    jc, jl = j // 4, j % 4
                    if parked is not None and j in parked:
                        pt = parked[j]
                    else:
                        pt, ps_s = score_exp(qb, pr, entry, ptp)
                    for hh, h in enumerate(heads):
                        nc.tensor.matmul(
                            po[h][:, off:QB],
                            lhsT=V_c[jc][:, jl, h, :],
                            rhs=pt[:, hh, off:QB],
                            start=(j == first_j),
                            stop=(j == last_j),
                        )
                # normalize: 1/l per head, replicate across 64 partitions via
                # a K=1 indicator matmul into the just-consumed last scores
                # tile (avoids a fresh psum slot at the pair boundary).
                # The very last pair routes its copies through ACT, which is
                # idle once the final exps drain, keeping the tail off DVE.
                tail = qb == 3 and pr == 1
                if ps_s is None:
                    ps_s = psum.tile([128, 2, QB], f32, name="ps_s", tag="ss")
                ps_rl = ps_s[:, 0, :]
                l2 = small.tile([1, 2, QB], f32, name="l2", tag="l2")
                for hh, h in enumerate(heads):
                    if tail:
                        nc.scalar.activation(
                            l2[:, hh, :], po[h][DK : DK + 1, :], AF.Copy
                        )
                    else:
                        nc.vector.tensor_copy(
                            out=l2[:, hh, :], in_=po[h][DK : DK + 1, :]
                        )
                rl2 = small.tile([1, 2, QB], f32, name="rl2", tag="rl2")
                nc.vector.reciprocal_approx_fast(out=rl2, in_=l2)
                for hh in range(2):
                    nc.tensor.matmul(
                        ps_rl[64 * hh : 64 * hh + 64, :],
                        lhsT=ones64_sb,
                        rhs=rl2[:, hh, :],
                        start=True,
                        stop=True,
                    )
                rl_bc = small.tile([128, QB], f32, name="rl_bc", tag="rl_bc")
                if tail:
                    nc.scalar.activation(rl_bc, ps_rl, AF.Copy)
                else:
                    nc.vector.tensor_copy(out=rl_bc, in_=ps_rl)
                for hh, h in enumerate(heads):
                    p0 = 64 * hh
                    nc.vector.tensor_tensor(
                        out=OT_sb[p0 : p0 + 64, h // 2, :],
                        in0=po[h][0:DK, :],
                        in1=rl_bc[p0 : p0 + 64, :],
                        op=AO.mult,
                    )

            def outproj(qb, OT_sb, on_act=False, ytag="pp"):
                for ot8 in range(8):
                    ps_y = psum.tile([128, QB], f32, name="ps_y", tag=ytag)
                    for ct in range(2):
                        nc.tensor.matmul(
                            ps_y,
                            lhsT=wo_sb[:, ct, 128 * ot8 : 128 * (ot8 + 1)],
                            rhs=OT_sb[:, ct, :],
                            start=(ct == 0),
                            stop=(ct == 1),
                        )
                    ysb = ystage.tile([128, QB], f16, name="ysb")
                    if on_act:
                        nc.scalar.activation(ysb, ps_y, AF.Copy)
                    else:
                        nc.vector.tensor_copy(out=ysb, in_=ps_y)
                    dma(
                        out=yT_d[
                            128 * ot8 : 128 * (ot8 + 1),
                            QB * qb : QB * (qb + 1),
                        ],
                        in_=ysb,
                    )

            # ---- emission order ----
            # chunk 0 criticals first: wq slices interleaved with q-chunk0
            # inputs, then the QK/V projections; remaining weights ride along.
            OTm = {}
            for qb in range(4):
                OTm[qb] = ot_pool.tile(
                    [128, 2, QB], st_dt, name=f"OT{qb}", tag="OT"
                )
            for i in range(4):
                nc.vector.memset(V_c[i][:, :, :, DK : DK + 1], 1.0)
            nc.vector.memset(ones64_sb, 1.0)
            dma(out=bq_sb, in_=bq_d)
            dma(out=bk_sb, in_=bk_d)

            # chunk-0: the head is DMA-ISSUE-bound (~600ns per descriptor on
            # one queue), so split issues across the two HWDGE engines: the
            # q path on sync, the k/v paths on the (still idle) scalar engine
            q0x, k0x, v0x = [], [], []
            for dm in range(NDM):
                load_w(wq_sb, wqT_d, dm)
                load_w(wk_sb, wkT_d, dm, eng=dma_act)
                q0x.append(stream_x(qT_d, 0, dm, "xt"))
                k0x.append(stream_x(kT_d, 0, dm, "xt", eng=dma_act))
            proj_qk(0, qT_d, wq_sb, bq_sb, QT_c[0], xts=q0x)
            for dm in range(NDM):
                load_w(wv_sb, wvT_d, dm, eng=dma_act)
                v0x.append(stream_x(vT_d, 0, dm, "xt", eng=dma_act))
            dma(out=bias_sb, in_=bias_d.rearrange("n p o -> p n o"))
            proj_qk(0, kT_d, wk_sb, bk_sb, KT_c[0], xts=k0x)
            proj_v(0, xts=v0x)
            for half in range(2):
                dma_act(
                    out=wo_sb[:, half, :],
                    in_=woT_d[128 * half : 128 * (half + 1), :],
                )

            # later q-blocks have the most exp (ACT) work and the least
            # natural PE filler: projections and output projections are
            # pushed late, and the qb=3 score/exp work that needs no fresh
            # projection is parked early so the finish is PE-bound
            attn_pair(0, 0, OTm[0])
            proj_qk(1, qT_d, wq_sb, bq_sb, QT_c[1])
            attn_pair(0, 1, OTm[0])
            proj_qk(1, kT_d, wk_sb, bk_sb, KT_c[1])
            proj_v(1)
            attn_pair(1, 0, OTm[1])
            proj_qk(2, qT_d, wq_sb, bq_sb, QT_c[2])
            attn_pair(1, 1, OTm[1])
            proj_qk(2, kT_d, wk_sb, bk_sb, KT_c[2])
            proj_v(2)
            attn_pair(2, 0, OTm[2])
            outproj(0, OTm[0])
            proj_qk(3, qT_d, wq_sb, bq_sb, QT_c[3])
            park30 = attn_park(3, 0, jmax=11)
            attn_pair(2, 1, OTm[2])
            proj_qk(3, kT_d, wk_sb, bk_sb, KT_c[3])
            park31 = attn_park(3, 1, jmax=11)
            proj_v(3)
            attn_pair(3, 0, OTm[3], parked=park30)
            outproj(1, OTm[1])
            attn_pair(3, 1, OTm[3], parked=park31)
            outproj(2, OTm[2], on_act=True)
            # final output block: all psum is dead, so spread it over 8
            # distinct bank regions and emit every ct0 matmul before the
            # ct1s — the in-order PE queue then never stalls a ready ct0
            # behind a ct1 waiting on the last normalize
            y3 = [
                psum.tile([128, 2, QB], f32, name="y3a", tag="ss"),
                psum.tile([128, 2, QB], f32, name="y3b", tag="ss"),
                psum.tile([128, QB], f32, name="y3c", tag="po"),
                psum.tile([128, QB], f32, name="y3d", tag="po"),
                psum.tile([128, QB], f32, name="y3e", tag="pp"),
                psum.tile([128, QB], f32, name="y3f", tag="pp"),
            ]
            yreg = [
                y3[0][:, 0, :], y3[0][:, 1, :], y3[1][:, 0, :],
                y3[1][:, 1, :], y3[2], y3[3], y3[4], y3[5],
            ]
            for ct in range(2):
                for ot8 in range(8):
                    nc.tensor.matmul(
                        yreg[ot8],
                        lhsT=wo_sb[:, ct, 128 * ot8 : 128 * (ot8 + 1)],
                        rhs=OTm[3][:, ct, :],
                        start=(ct == 0),
                        stop=(ct == 1),
                    )
            for ot8 in range(8):
                ysb = ystage.tile([128, QB], f16, name="ysb")
                nc.scalar.activation(ysb, yreg[ot8], AF.Copy)
                dma(
                    out=yT_d[128 * ot8 : 128 * (ot8 + 1), QB * 3 : QB * 4],
                    in_=ysb,
                )

    nc.compile()
    return nc


def _get_nc(mode, plan, n_bias, key):
    ck = (mode, key, n_bias)
    if ck not in _CACHE:
        _CACHE[ck] = _build(mode, plan, n_bias)
    return _CACHE[ck]


def _prep_inputs(q, k, v, wq, bq, wk, bk, wv, wo, bias_pack, mode):
    """Build the 8 per-core input maps."""
    f32 = np.float32
    if mode == "bf16":
        import ml_dtypes

        io_np = ml_dtypes.bfloat16
    else:
        io_np = f32

    wqT = np.ascontiguousarray(np.asarray(wq, f32).T)
    wkT = np.ascontiguousarray(np.asarray(wk, f32).T)
    wvT = np.ascontiguousarray(np.asarray(wv, f32).T)
    woT = np.ascontiguousarray(np.asarray(wo, f32).T)

    in_maps = []
    for c in range(NCORES):
        b, g = c // GROUPS, c % GROUPS
        sl = slice(GDIM * g, GDIM * (g + 1))
        im = {
            "qT": np.ascontiguousarray(np.asarray(q[b], f32).T).astype(io_np),
            "kT": np.ascontiguousarray(np.asarray(k[b], f32).T).astype(io_np),
            "vT": np.ascontiguousarray(np.asarray(v[b], f32).T).astype(io_np),
            "wqT": np.ascontiguousarray(wqT[:, sl]).astype(io_np),
            "wkT": np.ascontiguousarray(wkT[:, sl]).astype(io_np),
            "wvT": np.ascontiguousarray(wvT[:, sl]).astype(io_np),
            "woT": np.ascontiguousarray(woT[sl, :]).astype(io_np),
            "bq2": np.ascontiguousarray(
                np.asarray(bq, f32)[sl].reshape(2, 128).T
            ),
            "bk2": np.ascontiguousarray(
                np.asarray(bk, f32)[sl].reshape(2, 128).T
            ),
            "bias_pack": bias_pack,
        }
        in_maps.append(im)
    return in_maps


def _kernel_impl(q, k, v, mask, wq, bq, wk, bk, wv, bv, wo, bo, trace=False):
    from concourse.bass_utils import run_bass_kernel_spmd

    f32 = np.float32
    m2d = np.asarray(mask)[0, 0]
    plan, bias_pack, key = _make_plan(m2d)
    nc = _get_nc(MODE, plan, bias_pack.shape[0], key)
    in_maps = _prep_inputs(q, k, v, wq, bq, wk, bk, wv, wo, bias_pack, MODE)

    res = run_bass_kernel_spmd(nc, in_maps, list(range(NCORES)), trace=trace)

    bo_eff = (
        np.asarray(bo, np.float64)
        + np.asarray(bv, np.float64) @ np.asarray(wo, np.float64).T
    ).astype(f32)

    out = np.zeros((B, S, D), f32)
    for c in range(NCORES):
        out[c // GROUPS] += res.results[c]["yT"].astype(f32).T
    out += bo_eff
    return out, res


def kernel(q, k, v, mask, wq, bq, wk, bk, wv, bv, wo, bo):
    out, _ = _kernel_impl(q, k, v, mask, wq, bq, wk, bk, wv, bv, wo, bo)
    return out
